# revision 24
# baseline (speedup 1.0000x reference)
"""Trainium2 Bass kernel for the NodeAttentionLayer (GAT-style) problem.

Math (per reference.py):
    h_t = t_input @ W_t; h_o = o_input @ W_o
    s_t = h_t @ a[:F];  s_o = h_o @ a[F:]
    e[i,j]   = leaky_relu(s_t[i] + s_o[j], 0.2)
    att      = softmax(where(adj>0, e, -9e15), axis=1)
    out      = elu(att @ h_o)

Sorted-split identity: with c = (s_t[i]+s_o[j] > 0), v1 = exp(s_o), v2 =
exp(0.2 s_o), r = exp(0.8 s_t):
    att-numerator @ [h_o|1] = r[i] * (W1 @ M1) + (W2 @ M2)
where W1 = v1*[h_o|1], W2 = v2*[h_o|1], M1 = adj*c, M2 = adj - M1; the
ones column carries the softmax denominator; softmax max-trick cancels.

The host permutes j by ascending s_o and deals i by descending s_t rank
round-robin across the 8 cores (permutation-invariant math; output rows
un-permuted on host).  Then for every 128-j tile T, c[:,i] is all-ones
for i < A_T, all-zeros for i >= B_T, and mixed only on a narrow slab
[A_T, B_T) (~20 cols).  Prefix columns feed the W1 stream and suffix
columns the W2 stream DIRECTLY from adj (no mask work); only the slab
needs c / M1 / M2 element ops.  Each i-column crosses the PE once per
j-tile instead of twice, and the DVE/ACT mask work drops ~40x.

adj and o ship as fp8e4 (adj 0/1 is exact in fp8; mixed bf16-stationary
x fp8-moving matmul verified exact on HW), halving the dominant DMA.

Split points A_T/B_T are data-dependent; the Bass program is built per
input (compile happens inside kernel(), cached on the split tuple).
Cores share one SPMD program: A_T = min over cores, B_T = max.
"""

import contextlib
import ctypes
import sys
import types

import ml_dtypes
import numpy as np

import concourse.bass as bass
import concourse.mybir as mybir
import concourse.tile as tile
from concourse.vector_clock import ScopedClock

bf16 = ml_dtypes.bfloat16
f8e4 = ml_dtypes.float8_e4m3

# ---------------------------------------------------------------------------
# Environment shims (same as baseline)
# ---------------------------------------------------------------------------

def _patch_tile_drain():
    if getattr(tile.TileContext, "_drain_patch_installed", False):
        return

    def _drain_and_barrier(self, tick_clock, wait_clock):
        nop_inst = self.nc.sync.nop(nofuse=True)
        wait_clock.add_sem_waits(
            nop_inst.ins, ScopedClock({None: tick_clock.global_clock})
        )
        ow = list(nop_inst.ins.sync_info.on_wait) if nop_inst.ins.sync_info else []
        if len(ow) > 1:
            nop_inst.ins.sync_info.on_wait = ow[:1]
            for w in ow[1:]:
                extra = self.nc.sync.nop(nofuse=True)
                if extra.ins.sync_info is None:
                    extra.ins.sync_info = mybir.SyncInfo(on_wait=[w], on_update=[])
                else:
                    extra.ins.sync_info.on_wait = [w]
        self.nc.sync.drain()
        self.nc.all_engine_barrier()
        popped = self.nc._tile_sem_poison_stack.pop()
        assert popped is self._sem_poison
        self.nc.clear_and_free_semaphores(list(self.sems.allocated().values()))
        self.nc.all_engine_barrier()

    tile.TileContext._drain_and_barrier = _drain_and_barrier
    tile.TileContext._drain_patch_installed = True


def _install_ntff_hook():
    if "antenv.axon_hooks" in sys.modules:
        return
    import antenv

    state = {"hook": None}
    mod = types.ModuleType("antenv.axon_hooks")
    mod.set_axon_ntff_profile_hook = lambda h: state.__setitem__("hook", h)
    mod.get_axon_ntff_profile_hook = lambda: state["hook"]
    sys.modules["antenv.axon_hooks"] = mod
    antenv.axon_hooks = mod

    try:
        lib = ctypes.CDLL("/opt/axon/libaxon_pjrt.so")
    except OSError:
        return
    if not hasattr(lib, "axon_start_nrt_profile"):
        return
    lib.axon_start_nrt_profile.argtypes = [
        ctypes.POINTER(ctypes.c_int64),
        ctypes.c_size_t,
    ]
    lib.axon_start_nrt_profile.restype = ctypes.c_int64
    lib.axon_stop_nrt_profile.argtypes = [ctypes.c_char_p]
    lib.axon_stop_nrt_profile.restype = ctypes.c_int64

    @contextlib.contextmanager
    def _ntff_hook(output_dir, device_ids):
        import jax

        jax.devices()
        if device_ids:
            ids = (ctypes.c_int64 * len(device_ids))(*device_ids)
            rc = lib.axon_start_nrt_profile(ids, len(device_ids))
        else:
            rc = lib.axon_start_nrt_profile(None, 0)
        if rc != 0:
            raise RuntimeError(f"axon_start_nrt_profile rc={rc}")
        try:
            yield
        finally:
            n = lib.axon_stop_nrt_profile(str(output_dir).encode())
            print(f"profile: {n} file(s) written to {output_dir}", file=sys.stderr)

    state["hook"] = _ntff_hook


_patch_tile_drain()
_install_ntff_hook()

# Walrus disables its LDWEIGHTS optimizer by default; each self-loading
# matmul then pays its weight-load serially (~90ns). Flipping the flag lets
# consecutive matmuls overlap weight loads. Toggleable for A/B testing.
LDW_OPT = False   # --enable-ldw-opt=true breaks walrus visitInstLdweights


def _install_ldw_opt_patch():
    import concourse.bass_utils as _bu

    if getattr(_bu, "_ldw_opt_patch", False):
        return
    _orig = _bu.run_command

    def _patched(cmd, *args, **kw):
        if LDW_OPT and isinstance(cmd, list):
            cmd = ["--enable-ldw-opt=true" if c == "--enable-ldw-opt=false" else c
                   for c in cmd]
        return _orig(cmd, *args, **kw)

    _bu.run_command = _patched
    _bu._ldw_opt_patch = True


_install_ldw_opt_patch()


def _split_multi_waits(nc):
    import bass_rust

    k = 0
    for f in nc.m.functions:
        for blk in f.blocks:
            insts = blk.instructions
            out = []
            changed = False
            for inst in insts:
                si = inst.sync_info
                ow = list(si.on_wait) if si is not None else []
                if len(ow) > 1:
                    for w in ow[:-1]:
                        nop = bass_rust.InstNoOp(
                            name=f"waitsplit-{k}", engine=inst.engine
                        )
                        k += 1
                        nop.sync_info = mybir.SyncInfo(on_wait=[w], on_update=[])
                        out.append(nop)
                    si.on_wait = [ow[-1]]
                    changed = True
                out.append(inst)
            if changed:
                blk.instructions = out


# ---------------------------------------------------------------------------
# Problem constants
# ---------------------------------------------------------------------------
N_T, N_O, F_IN, F_OUT = 8192, 8192, 256, 64
N_CORES = 8
R = N_T // N_CORES            # 1024 t-rows per core
NJ = N_O // 128               # 64 j-tiles
NQ = NJ // 4                  # 16 quads (adj DMA batches of 4 tiles)
KC = F_IN // 128              # 2 contraction chunks
NG = 4                        # o-side groups
GT = NJ // NG                 # 16 j-tiles per group
F32 = mybir.dt.float32
BF16 = mybir.dt.bfloat16
FP8 = mybir.dt.float8e4
AF = mybir.ActivationFunctionType
OP = mybir.AluOpType


def _rep_free(ap, reps):
    """View [P, K] tile as [P, K, reps] via innermost step-0."""
    return bass.AP(tensor=ap.tensor, offset=ap.offset, ap=list(ap.ap) + [[0, reps]])


def build_kernel(A, B, SW, W_S, split_waits=True):
    """A, B: length-64 int tuples; j-tile T uses raw adj on i in [0,A_T)
    (stream 1) and [B_T,1024) (stream 2); masked slab on [A_T,B_T).
    SW: per-tile start of the W_S-wide adj window ([S_T, S_T+W_S) covers
    the slab); S is piecewise-arithmetic in T so the window loads in a
    handful of constant-stride DMAs."""
    assert W_S <= 128

    nc = bass.Bass("TRN2")

    t_T = nc.dram_tensor("t_T", [F_IN, R], BF16, kind="ExternalInput")
    o_T = nc.dram_tensor("o_T", [F_IN, N_O], BF16, kind="ExternalInput")
    wt_d = nc.dram_tensor("wt_d", [F_IN, F_OUT], BF16, kind="ExternalInput")
    wo_d = nc.dram_tensor("wo_d", [F_IN, F_OUT], BF16, kind="ExternalInput")
    a_d = nc.dram_tensor("a_d", [2 * F_OUT, 1], BF16, kind="ExternalInput")
    adjT = nc.dram_tensor("adjT", [N_O, R], FP8, kind="ExternalInput")
    out_d = nc.dram_tensor("out_d", [F_OUT, R], F32, kind="ExternalOutput")

    st_dram = nc.dram_tensor("st_dram", [1, R], BF16, kind="Internal")
    hoT_dram_g = [nc.dram_tensor(f"hoT_dram{g}", [80, 2048], BF16,
                                 kind="Internal") for g in range(NG)]

    with tile.TileContext(nc) as tc, contextlib.ExitStack() as ctx:
        S = ctx.enter_context(tc.tile_pool(name="singles", bufs=1))
        adj_pool = ctx.enter_context(tc.tile_pool(name="adj", bufs=4))
        acc = ctx.enter_context(tc.tile_pool(name="acc", bufs=1, space="PSUM"))
        mps = ctx.enter_context(tc.tile_pool(name="mps", bufs=2, space="PSUM"))
        hop = ctx.enter_context(tc.tile_pool(name="hop", bufs=2, space="PSUM"))

        # ------------------------------------------------------------------
        # t-side head
        # ------------------------------------------------------------------
        t_sb = S.tile([128, KC, R], BF16)
        nc.sync.dma_start(
            out=t_sb[:, :, :],
            in_=bass.AP(tensor=t_T, offset=0,
                        ap=[[R, 128], [128 * R, KC], [1, R]]))
        wt_sb = S.tile([128, KC, F_OUT], BF16)
        nc.sync.dma_start(
            out=wt_sb[:, :, :],
            in_=bass.AP(tensor=wt_d, offset=0,
                        ap=[[F_OUT, 128], [128 * F_OUT, KC], [1, F_OUT]]))
        wo_sb = S.tile([128, KC, F_OUT], BF16)
        nc.sync.dma_start(
            out=wo_sb[:, :, :],
            in_=bass.AP(tensor=wo_d, offset=0,
                        ap=[[F_OUT, 128], [128 * F_OUT, KC], [1, F_OUT]]))
        a_t_b = S.tile([128, F_OUT], BF16)
        nc.sync.dma_start(
            out=a_t_b[:, :],
            in_=bass.AP(tensor=a_d, offset=0, ap=[[0, 128], [1, F_OUT]]))
        a_o_b = S.tile([128, F_OUT], BF16)
        nc.sync.dma_start(
            out=a_o_b[:, :],
            in_=bass.AP(tensor=a_d, offset=F_OUT, ap=[[0, 128], [1, F_OUT]]))

        # w065 = [W_o | W_o @ a_o]  (projects o -> [h_o | s_o])
        w065 = S.tile([128, KC, F_OUT + 1], BF16)
        nc.vector.tensor_copy(w065[:, :, 0:F_OUT], wo_sb[:, :, :])
        prod_ao = S.tile([128, KC, F_OUT], F32)
        ao_ap = a_o_b[:, :]
        nc.vector.tensor_tensor(
            prod_ao[:, :, :], wo_sb[:, :, :],
            bass.AP(tensor=ao_ap.tensor, offset=ao_ap.offset,
                    ap=[list(ao_ap.ap[0]), [0, KC], [1, F_OUT]]),
            OP.mult)
        woa_f = S.tile([128, KC], F32)
        nc.vector.tensor_reduce(woa_f[:, :], prod_ao[:, :, :],
                                mybir.AxisListType.X, OP.add)
        nc.vector.tensor_copy(w065[:, :, F_OUT], woa_f[:, :])

        # wta = W_t @ a_t (so s_t = t @ wta without materializing h_t)
        prod_at = S.tile([128, KC, F_OUT], F32)
        at_ap = a_t_b[:, :]
        nc.vector.tensor_tensor(
            prod_at[:, :, :], wt_sb[:, :, :],
            bass.AP(tensor=at_ap.tensor, offset=at_ap.offset,
                    ap=[list(at_ap.ap[0]), [0, KC], [1, F_OUT]]),
            OP.mult)
        wta_f = S.tile([128, KC], F32)
        nc.vector.tensor_reduce(wta_f[:, :], prod_at[:, :, :],
                                mybir.AxisListType.X, OP.add)
        wta_b = S.tile([128, KC], BF16)
        nc.vector.tensor_copy(wta_b[:, :], wta_f[:, :])

        # zero moving buffer: 4 full-width start=True matmuls clear the
        # accumulator banks' has_written for the whole bank (robust across
        # repeated NEFF executions with partial-coverage accumulation).
        # Dependency-free (memset weights) so they fire early and warm HAM.
        t1_acc = [acc.tile([F_OUT + 1, 512], F32, tag=f"t1_{n}", name=f"t1_{n}")
                  for n in range(2)]
        t2_acc = [acc.tile([F_OUT + 1, 512], F32, tag=f"t2_{n}", name=f"t2_{n}")
                  for n in range(2)]
        with tc.high_priority():
            zbuf = S.tile([128, 512], FP8)
            nc.vector.memset(zbuf[:, :], 0.0)
            zw = S.tile([128, F_OUT + 1], BF16)
            nc.vector.memset(zw[:, :], 0.0)
            for ps in (t1_acc[0], t1_acc[1], t2_acc[0], t2_acc[1]):
                nc.tensor.matmul(ps[:, :], zw[:, :], zbuf[:, :],
                                 start=True, stop=False)

        # s_t and r = exp(0.8 s_t)
        with tc.high_priority():
            st_b = S.tile([1, R], BF16)
            r_b = S.tile([1, R], BF16)
            for n in range(R // 512):
                st_ps = mps.tile([1, 512], F32, tag="mps")
                for c in range(KC):
                    nc.tensor.matmul(st_ps[:, :], wta_b[:, c:c + 1],
                                     t_sb[:, c, n * 512:(n + 1) * 512],
                                     start=(c == 0), stop=(c == KC - 1))
                nc.vector.tensor_copy(st_b[:, n * 512:(n + 1) * 512], st_ps[:, :])
                nc.scalar.activation(r_b[:, n * 512:(n + 1) * 512], st_ps[:, :],
                                     AF.Exp, scale=0.8)
            nc.sync.dma_start(out=st_dram[:, :], in_=st_b[0:1, :])
            st_bcast = S.tile([128, R], BF16)
            nc.sync.dma_start(
                out=st_bcast[:, :],
                in_=bass.AP(tensor=st_dram, offset=0, ap=[[0, 128], [1, R]]),
            )

        # ------------------------------------------------------------------
        # o-side, group-pipelined: h_oT, s_o, transpose, W-build, slabs
        # ------------------------------------------------------------------
        ho_g_t = [S.tile([128, GT, 80], BF16, name=f"hog{g}") for g in range(NG)]
        nso_g = [S.tile([128, GT], F32, name=f"nsog{g}") for g in range(NG)]
        v1_g = [S.tile([128, GT], BF16, name=f"v1g{g}") for g in range(NG)]
        v2_g = [S.tile([128, GT], BF16, name=f"v2g{g}") for g in range(NG)]
        w1_g = [S.tile([128, GT, F_OUT + 1], BF16, name=f"w1g{g}")
                for g in range(NG)]
        w2_g = [S.tile([128, GT, F_OUT + 1], BF16, name=f"w2g{g}")
                for g in range(NG)]
        adjs = S.tile([128, NJ, W_S], FP8, name="adjs")
        c_g = [S.tile([128, GT, W_S], BF16, name=f"cg{g}") for g in range(NG)]
        m1_g = [S.tile([128, GT, W_S], BF16, name=f"m1g{g}") for g in range(NG)]
        m2_g = [S.tile([128, GT, W_S], BF16, name=f"m2g{g}") for g in range(NG)]

        # o DMAs upfront (4 live buffers); adj slab window split per group
        # into constant-stride runs (few DMA instructions, ~2us of
        # descriptor-gen each, interleaved so no single 8us ring block)
        o_gs = []
        for g in range(NG):
            o_g = S.tile([128, KC, 2048], BF16, name=f"og{g}")
            nc.scalar.dma_start(
                out=o_g[:, :, :],
                in_=bass.AP(tensor=o_T, offset=g * 2048,
                            ap=[[N_O, 128], [128 * N_O, KC], [1, 2048]]))
            o_gs.append(o_g)
            t_lo, t_hi = g * GT, (g + 1) * GT
            T0 = t_lo
            while T0 < t_hi:
                if T0 == t_hi - 1:
                    runs = (T0, t_hi, 0)
                else:
                    d = SW[T0 + 1] - SW[T0]
                    T1 = T0 + 1
                    while T1 + 1 < t_hi and SW[T1 + 1] - SW[T1] == d:
                        T1 += 1
                    runs = (T0, T1 + 1, d)
                lo_t, hi_t, d = runs
                nc.scalar.dma_start(
                    out=adjs[:, lo_t:hi_t, :],
                    in_=bass.AP(tensor=adjT, offset=(128 * lo_t) * R + SW[lo_t],
                                ap=[[R, 128], [128 * R + d, hi_t - lo_t],
                                    [1, W_S]]))
                T0 = hi_t

        hoT_bufs = [S.tile([80, 2048], BF16, name=f"hoth{k}") for k in range(2)]
        for g in range(NG):
            o_g = o_gs[g]
            hoT_g = hoT_bufs[g % 2]
            for ch in range(4):
                hps = hop.tile([F_OUT + 1, 512], F32, tag="hop",
                               name=f"hop{g}_{ch}")
                for c in range(KC):
                    nc.tensor.matmul(
                        hps[:, :], w065[:, c, :],
                        o_g[:, c, ch * 512:(ch + 1) * 512],
                        start=(c == 0), stop=(c == KC - 1))
                nc.scalar.copy(
                    hoT_g[0:F_OUT + 1, ch * 512:(ch + 1) * 512], hps[:, :])
            nc.scalar.dma_start(out=hoT_dram_g[g][:, :], in_=hoT_g[:, :])
            nc.scalar.dma_start_transpose(ho_g_t[g][:, :, :], hoT_dram_g[g][:, :])

            so_view = ho_g_t[g][:, :, F_OUT]
            nc.vector.tensor_scalar(nso_g[g][:, :], so_view, -1.0, None, OP.mult)
            nc.scalar.activation(v1_g[g][:, :], so_view, AF.Exp)
            nc.scalar.activation(v2_g[g][:, :], so_view, AF.Exp, scale=0.2)

            # W-build
            nc.vector.tensor_tensor(
                w1_g[g][:, :, 0:F_OUT], ho_g_t[g][:, :, 0:F_OUT],
                _rep_free(v1_g[g][:, :], F_OUT), OP.mult)
            nc.vector.tensor_copy(w1_g[g][:, :, F_OUT], v1_g[g][:, :])
            nc.vector.tensor_tensor(
                w2_g[g][:, :, 0:F_OUT], ho_g_t[g][:, :, 0:F_OUT],
                _rep_free(v2_g[g][:, :], F_OUT), OP.mult)
            nc.vector.tensor_copy(w2_g[g][:, :, F_OUT], v2_g[g][:, :])

            # slab prep: c = (s_t > -s_o) on [A,B) inside the window; M1; M2
            nc.vector.memset(c_g[g][:, :, :], 0.0)
            for u in range(GT):
                T = g * GT + u
                w = B[T] - A[T]
                if w <= 0:
                    continue
                off = A[T] - SW[T]
                nc.vector.tensor_scalar(
                    c_g[g][:, u, off:off + w], st_bcast[:, A[T]:B[T]],
                    nso_g[g][:, u:u + 1], None, OP.is_gt)
            gsl = slice(g * GT, (g + 1) * GT)
            nc.vector.tensor_tensor(m1_g[g][:, :, :], c_g[g][:, :, :],
                                    adjs[:, gsl, :], OP.mult)
            nc.vector.tensor_tensor(m2_g[g][:, :, :], adjs[:, gsl, :],
                                    m1_g[g][:, :, :], OP.subtract)

        # ------------------------------------------------------------------
        # Main: per j-tile, prefix/suffix from raw adj + slab from M1/M2
        # ------------------------------------------------------------------
        # pre-pass: find last matmul per (stream, bank) for stop flags
        def tile_ranges(T):
            s1 = [(0, A[T], "adj"), (A[T], B[T], "m1")]
            s2 = [(A[T], B[T], "m2"), (B[T], R, "adj")]
            out = {1: [], 2: []}
            for stream, ranges in ((1, s1), (2, s2)):
                for lo, hi, src in ranges:
                    for n in range(2):
                        l = max(lo, n * 512)
                        h = min(hi, (n + 1) * 512)
                        if l < h:
                            out[stream].append((n, l, h, src))
            return out

        last_mm = {}
        for T in range(NJ):
            for stream, mms in tile_ranges(T).items():
                for k, (n, l, h, src) in enumerate(mms):
                    last_mm[(stream, n)] = (T, k)

        for q in range(NQ):
            batch = adj_pool.tile([128, 4, R], FP8, tag="adj", name=f"adj{q}")
            nc.sync.dma_start(
                out=batch[:, :, :],
                in_=bass.AP(tensor=adjT, offset=q * 512 * R,
                            ap=[[R, 128], [128 * R, 4], [1, R]]),
            )
            for s in range(4):
                T = q * 4 + s
                g, u = T // GT, T % GT
                ranges = tile_ranges(T)
                for stream, wgt, accs, mpack in (
                        (1, w1_g[g], t1_acc, m1_g[g]),
                        (2, w2_g[g], t2_acc, m2_g[g])):
                    for k, (n, l, h, src) in enumerate(ranges[stream]):
                        if src == "adj":
                            rhs = batch[:, s, l:h]
                        else:
                            rhs = mpack[:, u, l - SW[T]:h - SW[T]]
                        stop = last_mm[(stream, n)] == (T, k)
                        nc.tensor.matmul(
                            accs[n][:, l - n * 512:h - n * 512],
                            wgt[:, u, :], rhs, start=False, stop=stop)

        # ------------------------------------------------------------------
        # Tail: H = r*T1 + T2; out = elu(H[:F]/H[F])
        # ------------------------------------------------------------------
        ones65 = S.tile([1, F_OUT + 1], BF16)
        nc.vector.memset(ones65[:, :], 1.0)

        h_sb = S.tile([F_OUT + 1, R], F32)
        for n in range(2):
            sl = slice(n * 512, (n + 1) * 512)
            rb_ps = mps.tile([F_OUT + 1, 512], F32, tag="mps")
            nc.tensor.matmul(rb_ps[:, :], ones65[:, :], r_b[:, sl],
                             start=True, stop=True)
            nc.vector.tensor_copy(h_sb[:, sl], rb_ps[:, :])
            nc.vector.tensor_tensor(h_sb[:, sl], h_sb[:, sl], t1_acc[n][:, :],
                                    OP.mult)
            nc.vector.tensor_tensor(h_sb[:, sl], h_sb[:, sl], t2_acc[n][:, :],
                                    OP.add)

        zr = S.tile([1, R], F32)
        nc.vector.reciprocal(zr[:, :], h_sb[F_OUT:F_OUT + 1, :])
        zr_b = S.tile([1, R], BF16)
        nc.vector.tensor_copy(zr_b[:, :], zr[:, :])

        ot_sb = S.tile([F_OUT, R], F32)
        for n in range(2):
            sl = slice(n * 512, (n + 1) * 512)
            zb_ps = mps.tile([F_OUT, 512], F32, tag="mps")
            nc.tensor.matmul(zb_ps[:, :], ones65[:, 0:F_OUT], zr_b[:, sl],
                             start=True, stop=True)
            nc.vector.tensor_tensor(ot_sb[:, sl], h_sb[0:F_OUT, sl], zb_ps[:, :],
                                    OP.mult)

        # elu(x) = max(x,0) - 1 + exp(min(x,0))
        mn_sb = S.tile([F_OUT, R], F32)
        nc.vector.tensor_scalar(mn_sb[:, :], ot_sb[:, :], 0.0, None, OP.min)
        nc.scalar.activation(mn_sb[:, :], mn_sb[:, :], AF.Exp)
        nc.vector.tensor_scalar(ot_sb[:, :], ot_sb[:, :], 0.0, -1.0, OP.max, OP.add)
        nc.vector.tensor_tensor(ot_sb[:, :], ot_sb[:, :], mn_sb[:, :], OP.add)
        nc.sync.dma_start(out=out_d[:, :], in_=ot_sb[:, :])

    if split_waits:
        _split_multi_waits(nc)
    return nc


_CACHED = {}


def _get_compiled(A, B, S, W_S):
    key = (tuple(A), tuple(B), tuple(S), W_S)
    if key not in _CACHED:
        _CACHED.clear()
        _CACHED[key] = build_kernel(A, B, S, W_S)
    return _CACHED[key]


def _fit_window(A, B):
    """Find W_S and per-tile starts S (piecewise-arithmetic, clipped to
    [0, R-W_S]) with [S_T, S_T+W_S) covering every slab [A_T, B_T)."""
    Ts = np.arange(NJ)
    best = None
    for alpha in range(0, 49):
        beta = int((A - alpha * Ts).min())
        W = int((B - alpha * Ts).max()) - beta
        if W > 120:
            continue
        W4 = max(8, ((W + 3) // 4) * 4)
        Sv = np.clip(alpha * Ts + beta, 0, R - W4)
        if ((Sv <= A) & (B <= Sv + W4)).all():
            if best is None or W4 < best[0]:
                best = (W4, Sv)
    if best is None:
        W4 = max(8, ((int((B - A).max()) + 3) // 4) * 4)
        Sv = np.clip(A, 0, R - W4)
        best = (W4, Sv)
    W_S, Sv = best
    return [int(x) for x in Sv], int(W_S)


def kernel(t_input, o_input, W_t, W_o, a, adj, _trace=False):
    from concourse.bass_utils import run_bass_kernel_spmd

    t_input = np.asarray(t_input, dtype=np.float32)
    o_input = np.asarray(o_input, dtype=np.float32)
    W_t = np.asarray(W_t, dtype=np.float32)
    W_o = np.asarray(W_o, dtype=np.float32)
    a = np.asarray(a, dtype=np.float32)
    adj = np.asarray(adj)

    # scheduling metadata: sort j by s_o ascending, deal i by descending
    # s_t global rank round-robin across cores (so per-core split counts
    # differ by at most 1; min/max over cores shared by the SPMD program)
    s_o = (o_input @ W_o) @ a[F_OUT:, 0]
    s_t = t_input @ (W_t @ a[:F_OUT, 0])
    jperm = np.argsort(s_o, kind="stable")
    ipg = np.argsort(-s_t, kind="stable")
    so_s = s_o[jperm]
    lo = so_s[0::128]
    hi = so_s[127::128]
    A = np.full(NJ, 1 << 30, dtype=np.int64)
    Bb = np.full(NJ, -(1 << 30), dtype=np.int64)
    for m in range(N_CORES):
        neg = -s_t[ipg[m::N_CORES]]          # ascending
        Am = np.searchsorted(neg, lo, side="left")
        Bm = np.searchsorted(neg, hi, side="left")
        A = np.minimum(A, Am)
        Bb = np.maximum(Bb, Bm)
    A = np.maximum(A - 1, 0)
    Bb = np.minimum(Bb + 1, R)
    A[0] = 0
    Bb[-1] = R
    Sw, W_S = _fit_window(A, Bb)
    A = [int(x) for x in A]
    Bb = [int(x) for x in Bb]

    wt_b = W_t.astype(bf16)
    wo_b = W_o.astype(bf16)
    a_b = a.astype(bf16)
    o_Tb = np.ascontiguousarray(o_input[jperm].T).astype(bf16)

    in_maps = []
    for m in range(N_CORES):
        rows = ipg[m::N_CORES]
        adj_m = adj[rows][:, jperm].astype(np.float32)
        in_maps.append(
            {
                "t_T": np.ascontiguousarray(t_input[rows].T).astype(bf16),
                "o_T": o_Tb,
                "wt_d": wt_b,
                "wo_d": wo_b,
                "a_d": a_b,
                "adjT": np.ascontiguousarray(adj_m.T).astype(f8e4),
            }
        )

    nc = _get_compiled(A, Bb, Sw, W_S)
    res = run_bass_kernel_spmd(
        nc, in_maps, core_ids=list(range(N_CORES)), trace=_trace
    )
    out = np.empty((N_T, F_OUT), dtype=np.float32)
    for m in range(N_CORES):
        out[ipg[m::N_CORES]] = res.results[m]["out_d"].T
    if _trace:
        kernel.last_exec_time_ns = res.exec_time_ns
        kernel.last_results = res
    return out


# revision 25
# speedup vs baseline: 1.2133x; 1.2133x over previous
"""Trainium2 Bass kernel for the NodeAttentionLayer (GAT-style) problem.

Math (per reference.py):
    h_t = t_input @ W_t; h_o = o_input @ W_o
    s_t = h_t @ a[:F];  s_o = h_o @ a[F:]
    e[i,j]   = leaky_relu(s_t[i] + s_o[j], 0.2)
    att      = softmax(where(adj>0, e, -9e15), axis=1)
    out      = elu(att @ h_o)

Sorted-split identity: with c = (s_t[i]+s_o[j] > 0), v1 = exp(s_o), v2 =
exp(0.2 s_o), r = exp(0.8 s_t):
    att-numerator @ [h_o|1] = r[i] * (W1 @ M1) + (W2 @ M2)
where W1 = v1*[h_o|1], W2 = v2*[h_o|1], M1 = adj*c, M2 = adj - M1; the
ones column carries the softmax denominator; softmax max-trick cancels.

The host permutes j by ascending s_o and deals i by descending s_t rank
round-robin across the 8 cores (permutation-invariant math; output rows
un-permuted on host).  Then for every 128-j tile T, c[:,i] is all-ones
for i < A_T, all-zeros for i >= B_T, and mixed only on a narrow slab
[A_T, B_T) (~20 cols).  Prefix columns feed the W1 stream and suffix
columns the W2 stream DIRECTLY from adj (no mask work); only the slab
needs c / M1 / M2 element ops.  Each i-column crosses the PE once per
j-tile instead of twice, and the DVE/ACT mask work drops ~40x.

adj and o ship as fp8e4 (adj 0/1 is exact in fp8; mixed bf16-stationary
x fp8-moving matmul verified exact on HW), halving the dominant DMA.

Split points A_T/B_T are data-dependent; the Bass program is built per
input (compile happens inside kernel(), cached on the split tuple).
Cores share one SPMD program: A_T = min over cores, B_T = max.
"""

import contextlib
import ctypes
import sys
import types

import ml_dtypes
import numpy as np

import concourse.bass as bass
import concourse.mybir as mybir
import concourse.tile as tile
from concourse.vector_clock import ScopedClock

bf16 = ml_dtypes.bfloat16
f8e4 = ml_dtypes.float8_e4m3

# ---------------------------------------------------------------------------
# Environment shims (same as baseline)
# ---------------------------------------------------------------------------

def _patch_tile_drain():
    if getattr(tile.TileContext, "_drain_patch_installed", False):
        return

    def _drain_and_barrier(self, tick_clock, wait_clock):
        nop_inst = self.nc.sync.nop(nofuse=True)
        wait_clock.add_sem_waits(
            nop_inst.ins, ScopedClock({None: tick_clock.global_clock})
        )
        ow = list(nop_inst.ins.sync_info.on_wait) if nop_inst.ins.sync_info else []
        if len(ow) > 1:
            nop_inst.ins.sync_info.on_wait = ow[:1]
            for w in ow[1:]:
                extra = self.nc.sync.nop(nofuse=True)
                if extra.ins.sync_info is None:
                    extra.ins.sync_info = mybir.SyncInfo(on_wait=[w], on_update=[])
                else:
                    extra.ins.sync_info.on_wait = [w]
        self.nc.sync.drain()
        self.nc.all_engine_barrier()
        popped = self.nc._tile_sem_poison_stack.pop()
        assert popped is self._sem_poison
        self.nc.clear_and_free_semaphores(list(self.sems.allocated().values()))
        self.nc.all_engine_barrier()

    tile.TileContext._drain_and_barrier = _drain_and_barrier
    tile.TileContext._drain_patch_installed = True


def _install_ntff_hook():
    if "antenv.axon_hooks" in sys.modules:
        return
    import antenv

    state = {"hook": None}
    mod = types.ModuleType("antenv.axon_hooks")
    mod.set_axon_ntff_profile_hook = lambda h: state.__setitem__("hook", h)
    mod.get_axon_ntff_profile_hook = lambda: state["hook"]
    sys.modules["antenv.axon_hooks"] = mod
    antenv.axon_hooks = mod

    try:
        lib = ctypes.CDLL("/opt/axon/libaxon_pjrt.so")
    except OSError:
        return
    if not hasattr(lib, "axon_start_nrt_profile"):
        return
    lib.axon_start_nrt_profile.argtypes = [
        ctypes.POINTER(ctypes.c_int64),
        ctypes.c_size_t,
    ]
    lib.axon_start_nrt_profile.restype = ctypes.c_int64
    lib.axon_stop_nrt_profile.argtypes = [ctypes.c_char_p]
    lib.axon_stop_nrt_profile.restype = ctypes.c_int64

    @contextlib.contextmanager
    def _ntff_hook(output_dir, device_ids):
        import jax

        jax.devices()
        if device_ids:
            ids = (ctypes.c_int64 * len(device_ids))(*device_ids)
            rc = lib.axon_start_nrt_profile(ids, len(device_ids))
        else:
            rc = lib.axon_start_nrt_profile(None, 0)
        if rc != 0:
            raise RuntimeError(f"axon_start_nrt_profile rc={rc}")
        try:
            yield
        finally:
            n = lib.axon_stop_nrt_profile(str(output_dir).encode())
            print(f"profile: {n} file(s) written to {output_dir}", file=sys.stderr)

    state["hook"] = _ntff_hook


_patch_tile_drain()
_install_ntff_hook()

# Walrus disables its LDWEIGHTS optimizer by default; each self-loading
# matmul then pays its weight-load serially (~90ns). Flipping the flag lets
# consecutive matmuls overlap weight loads. Toggleable for A/B testing.
LDW_OPT = False   # --enable-ldw-opt=true breaks walrus visitInstLdweights


def _install_ldw_opt_patch():
    import concourse.bass_utils as _bu

    if getattr(_bu, "_ldw_opt_patch", False):
        return
    _orig = _bu.run_command

    def _patched(cmd, *args, **kw):
        if LDW_OPT and isinstance(cmd, list):
            cmd = ["--enable-ldw-opt=true" if c == "--enable-ldw-opt=false" else c
                   for c in cmd]
        return _orig(cmd, *args, **kw)

    _bu.run_command = _patched
    _bu._ldw_opt_patch = True


_install_ldw_opt_patch()


def _split_multi_waits(nc):
    import bass_rust

    k = 0
    for f in nc.m.functions:
        for blk in f.blocks:
            insts = blk.instructions
            out = []
            changed = False
            for inst in insts:
                si = inst.sync_info
                ow = list(si.on_wait) if si is not None else []
                if len(ow) > 1:
                    for w in ow[:-1]:
                        nop = bass_rust.InstNoOp(
                            name=f"waitsplit-{k}", engine=inst.engine
                        )
                        k += 1
                        nop.sync_info = mybir.SyncInfo(on_wait=[w], on_update=[])
                        out.append(nop)
                    si.on_wait = [ow[-1]]
                    changed = True
                out.append(inst)
            if changed:
                blk.instructions = out


# ---------------------------------------------------------------------------
# Problem constants
# ---------------------------------------------------------------------------
N_T, N_O, F_IN, F_OUT = 8192, 8192, 256, 64
N_CORES = 8
R = N_T // N_CORES            # 1024 t-rows per core
NJ = N_O // 128               # 64 j-tiles
NQ = NJ // 4                  # 16 quads (adj DMA batches of 4 tiles)
KC = F_IN // 128              # 2 contraction chunks
NG = 4                        # o-side groups
GT = NJ // NG                 # 16 j-tiles per group
F32 = mybir.dt.float32
BF16 = mybir.dt.bfloat16
FP8 = mybir.dt.float8e4
AF = mybir.ActivationFunctionType
OP = mybir.AluOpType


def _rep_free(ap, reps):
    """View [P, K] tile as [P, K, reps] via innermost step-0."""
    return bass.AP(tensor=ap.tensor, offset=ap.offset, ap=list(ap.ap) + [[0, reps]])


def build_kernel(A, B, SW, W_S, split_waits=True):
    """A, B: length-64 int tuples; j-tile T uses raw adj on i in [0,A_T)
    (stream 1) and [B_T,1024) (stream 2); masked slab on [A_T,B_T).
    SW: per-tile start of the W_S-wide adj window ([S_T, S_T+W_S) covers
    the slab); S is piecewise-arithmetic in T so the window loads in a
    handful of constant-stride DMAs."""
    assert W_S <= 128

    nc = bass.Bass("TRN2")

    t_T = nc.dram_tensor("t_T", [F_IN, R], BF16, kind="ExternalInput")
    o_T = nc.dram_tensor("o_T", [F_IN, N_O], BF16, kind="ExternalInput")
    wt_d = nc.dram_tensor("wt_d", [F_IN, F_OUT], BF16, kind="ExternalInput")
    wo_d = nc.dram_tensor("wo_d", [F_IN, F_OUT], BF16, kind="ExternalInput")
    a_d = nc.dram_tensor("a_d", [2 * F_OUT, 1], BF16, kind="ExternalInput")
    adjT = nc.dram_tensor("adjT", [N_O, R], FP8, kind="ExternalInput")
    out_d = nc.dram_tensor("out_d", [F_OUT, R], F32, kind="ExternalOutput")

    st_dram = nc.dram_tensor("st_dram", [1, R], BF16, kind="Internal")
    hoT_dram_g = [nc.dram_tensor(f"hoT_dram{g}", [80, 2048], BF16,
                                 kind="Internal") for g in range(NG)]

    with tile.TileContext(nc) as tc, contextlib.ExitStack() as ctx:
        S = ctx.enter_context(tc.tile_pool(name="singles", bufs=1))
        adj_pool = ctx.enter_context(tc.tile_pool(name="adj", bufs=4))
        acc = ctx.enter_context(tc.tile_pool(name="acc", bufs=1, space="PSUM"))
        mps = ctx.enter_context(tc.tile_pool(name="mps", bufs=2, space="PSUM"))
        hop = ctx.enter_context(tc.tile_pool(name="hop", bufs=2, space="PSUM"))

        # ------------------------------------------------------------------
        # t-side head
        # ------------------------------------------------------------------
        t_sb = S.tile([128, KC, R], BF16)
        nc.sync.dma_start(
            out=t_sb[:, :, :],
            in_=bass.AP(tensor=t_T, offset=0,
                        ap=[[R, 128], [128 * R, KC], [1, R]]))
        wt_sb = S.tile([128, KC, F_OUT], BF16)
        nc.sync.dma_start(
            out=wt_sb[:, :, :],
            in_=bass.AP(tensor=wt_d, offset=0,
                        ap=[[F_OUT, 128], [128 * F_OUT, KC], [1, F_OUT]]))
        wo_sb = S.tile([128, KC, F_OUT], BF16)
        nc.sync.dma_start(
            out=wo_sb[:, :, :],
            in_=bass.AP(tensor=wo_d, offset=0,
                        ap=[[F_OUT, 128], [128 * F_OUT, KC], [1, F_OUT]]))
        a_t_b = S.tile([128, F_OUT], BF16)
        nc.sync.dma_start(
            out=a_t_b[:, :],
            in_=bass.AP(tensor=a_d, offset=0, ap=[[0, 128], [1, F_OUT]]))
        a_o_b = S.tile([128, F_OUT], BF16)
        nc.sync.dma_start(
            out=a_o_b[:, :],
            in_=bass.AP(tensor=a_d, offset=F_OUT, ap=[[0, 128], [1, F_OUT]]))

        # w065 = [W_o | W_o @ a_o]  (projects o -> [h_o | s_o])
        w065 = S.tile([128, KC, F_OUT + 1], BF16)
        nc.vector.tensor_copy(w065[:, :, 0:F_OUT], wo_sb[:, :, :])
        prod_ao = S.tile([128, KC, F_OUT], F32)
        ao_ap = a_o_b[:, :]
        nc.vector.tensor_tensor(
            prod_ao[:, :, :], wo_sb[:, :, :],
            bass.AP(tensor=ao_ap.tensor, offset=ao_ap.offset,
                    ap=[list(ao_ap.ap[0]), [0, KC], [1, F_OUT]]),
            OP.mult)
        woa_f = S.tile([128, KC], F32)
        nc.vector.tensor_reduce(woa_f[:, :], prod_ao[:, :, :],
                                mybir.AxisListType.X, OP.add)
        nc.vector.tensor_copy(w065[:, :, F_OUT], woa_f[:, :])

        # wta = W_t @ a_t (so s_t = t @ wta without materializing h_t)
        prod_at = S.tile([128, KC, F_OUT], F32)
        at_ap = a_t_b[:, :]
        nc.vector.tensor_tensor(
            prod_at[:, :, :], wt_sb[:, :, :],
            bass.AP(tensor=at_ap.tensor, offset=at_ap.offset,
                    ap=[list(at_ap.ap[0]), [0, KC], [1, F_OUT]]),
            OP.mult)
        wta_f = S.tile([128, KC], F32)
        nc.vector.tensor_reduce(wta_f[:, :], prod_at[:, :, :],
                                mybir.AxisListType.X, OP.add)
        wta_b = S.tile([128, KC], BF16)
        nc.vector.tensor_copy(wta_b[:, :], wta_f[:, :])

        # zero moving buffer: 4 full-width start=True matmuls clear the
        # accumulator banks' has_written for the whole bank (robust across
        # repeated NEFF executions with partial-coverage accumulation).
        # Dependency-free (memset weights) so they fire early and warm HAM.
        t1_acc = [acc.tile([F_OUT + 1, 512], F32, tag=f"t1_{n}", name=f"t1_{n}")
                  for n in range(2)]
        t2_acc = [acc.tile([F_OUT + 1, 512], F32, tag=f"t2_{n}", name=f"t2_{n}")
                  for n in range(2)]
        with tc.high_priority():
            zbuf = S.tile([128, 512], FP8)
            nc.vector.memset(zbuf[:, :], 0.0)
            zw = S.tile([128, F_OUT + 1], BF16)
            nc.vector.memset(zw[:, :], 0.0)
            for ps in (t1_acc[0], t1_acc[1], t2_acc[0], t2_acc[1]):
                nc.tensor.matmul(ps[:, :], zw[:, :], zbuf[:, :],
                                 start=True, stop=False)

        # s_t and r = exp(0.8 s_t)
        with tc.high_priority():
            st_b = S.tile([1, R], BF16)
            r_b = S.tile([1, R], BF16)
            for n in range(R // 512):
                st_ps = mps.tile([1, 512], F32, tag="mps")
                for c in range(KC):
                    nc.tensor.matmul(st_ps[:, :], wta_b[:, c:c + 1],
                                     t_sb[:, c, n * 512:(n + 1) * 512],
                                     start=(c == 0), stop=(c == KC - 1))
                nc.vector.tensor_copy(st_b[:, n * 512:(n + 1) * 512], st_ps[:, :])
                nc.scalar.activation(r_b[:, n * 512:(n + 1) * 512], st_ps[:, :],
                                     AF.Exp, scale=0.8)
            nc.sync.dma_start(out=st_dram[:, :], in_=st_b[0:1, :])
            st_bcast = S.tile([128, R], BF16)
            nc.sync.dma_start(
                out=st_bcast[:, :],
                in_=bass.AP(tensor=st_dram, offset=0, ap=[[0, 128], [1, R]]),
            )

        # ------------------------------------------------------------------
        # o-side, group-pipelined: h_oT, s_o, transpose, W-build, slabs
        # ------------------------------------------------------------------
        ho_g_t = [S.tile([128, GT, 80], BF16, name=f"hog{g}") for g in range(NG)]
        nso_g = [S.tile([128, GT], F32, name=f"nsog{g}") for g in range(NG)]
        v1_g = [S.tile([128, GT], BF16, name=f"v1g{g}") for g in range(NG)]
        v2_g = [S.tile([128, GT], BF16, name=f"v2g{g}") for g in range(NG)]
        w1_g = [S.tile([128, GT, F_OUT + 1], BF16, name=f"w1g{g}")
                for g in range(NG)]
        w2_g = [S.tile([128, GT, F_OUT + 1], BF16, name=f"w2g{g}")
                for g in range(NG)]
        adjs = S.tile([128, NJ, W_S], FP8, name="adjs")
        c_g = [S.tile([128, GT, W_S], BF16, name=f"cg{g}") for g in range(NG)]
        m1_g = [S.tile([128, GT, W_S], BF16, name=f"m1g{g}") for g in range(NG)]
        m2_g = [S.tile([128, GT, W_S], BF16, name=f"m2g{g}") for g in range(NG)]

        # o DMAs upfront (4 live buffers); adj slab window split per group
        # into constant-stride runs (few DMA instructions, ~2us of
        # descriptor-gen each, interleaved so no single 8us ring block)
        o_gs = []
        for g in range(NG):
            o_g = S.tile([128, KC, 2048], BF16, name=f"og{g}")
            nc.scalar.dma_start(
                out=o_g[:, :, :],
                in_=bass.AP(tensor=o_T, offset=g * 2048,
                            ap=[[N_O, 128], [128 * N_O, KC], [1, 2048]]))
            o_gs.append(o_g)
        # adj slab window, constant-stride runs per group. On the SYNC ring:
        # its ~8us of descriptor-gen must not sit in front of the ACT copies
        # (scalar ring shares the sequencer between DMA-issue and compute).
        for g in range(NG):
            t_lo, t_hi = g * GT, (g + 1) * GT
            T0 = t_lo
            while T0 < t_hi:
                if T0 == t_hi - 1:
                    runs = (T0, t_hi, 0)
                else:
                    d = SW[T0 + 1] - SW[T0]
                    T1 = T0 + 1
                    while T1 + 1 < t_hi and SW[T1 + 1] - SW[T1] == d:
                        T1 += 1
                    runs = (T0, T1 + 1, d)
                lo_t, hi_t, d = runs
                nc.sync.dma_start(
                    out=adjs[:, lo_t:hi_t, :],
                    in_=bass.AP(tensor=adjT, offset=(128 * lo_t) * R + SW[lo_t],
                                ap=[[R, 128], [128 * R + d, hi_t - lo_t],
                                    [1, W_S]]))
                T0 = hi_t

        hoT_bufs = [S.tile([80, 2048], BF16, name=f"hoth{k}") for k in range(2)]
        for g in range(NG):
            o_g = o_gs[g]
            hoT_g = hoT_bufs[g % 2]
            for ch in range(4):
                hps = hop.tile([F_OUT + 1, 512], F32, tag="hop",
                               name=f"hop{g}_{ch}")
                for c in range(KC):
                    nc.tensor.matmul(
                        hps[:, :], w065[:, c, :],
                        o_g[:, c, ch * 512:(ch + 1) * 512],
                        start=(c == 0), stop=(c == KC - 1))
                nc.scalar.copy(
                    hoT_g[0:F_OUT + 1, ch * 512:(ch + 1) * 512], hps[:, :])
            nc.scalar.dma_start(out=hoT_dram_g[g][:, :], in_=hoT_g[:, :])
            nc.scalar.dma_start_transpose(ho_g_t[g][:, :, :], hoT_dram_g[g][:, :])

            so_view = ho_g_t[g][:, :, F_OUT]
            nc.vector.tensor_scalar(nso_g[g][:, :], so_view, -1.0, None, OP.mult)
            nc.scalar.activation(v1_g[g][:, :], so_view, AF.Exp)
            nc.scalar.activation(v2_g[g][:, :], so_view, AF.Exp, scale=0.2)

            # W-build
            nc.vector.tensor_tensor(
                w1_g[g][:, :, 0:F_OUT], ho_g_t[g][:, :, 0:F_OUT],
                _rep_free(v1_g[g][:, :], F_OUT), OP.mult)
            nc.vector.tensor_copy(w1_g[g][:, :, F_OUT], v1_g[g][:, :])
            nc.vector.tensor_tensor(
                w2_g[g][:, :, 0:F_OUT], ho_g_t[g][:, :, 0:F_OUT],
                _rep_free(v2_g[g][:, :], F_OUT), OP.mult)
            nc.vector.tensor_copy(w2_g[g][:, :, F_OUT], v2_g[g][:, :])

            # slab prep: c = (s_t > -s_o) on [A,B) inside the window; M1; M2
            nc.vector.memset(c_g[g][:, :, :], 0.0)
            for u in range(GT):
                T = g * GT + u
                w = B[T] - A[T]
                if w <= 0:
                    continue
                off = A[T] - SW[T]
                nc.vector.tensor_scalar(
                    c_g[g][:, u, off:off + w], st_bcast[:, A[T]:B[T]],
                    nso_g[g][:, u:u + 1], None, OP.is_gt)
            gsl = slice(g * GT, (g + 1) * GT)
            nc.vector.tensor_tensor(m1_g[g][:, :, :], c_g[g][:, :, :],
                                    adjs[:, gsl, :], OP.mult)
            nc.vector.tensor_tensor(m2_g[g][:, :, :], adjs[:, gsl, :],
                                    m1_g[g][:, :, :], OP.subtract)

        # ------------------------------------------------------------------
        # Main: per j-tile, prefix/suffix from raw adj + slab from M1/M2
        # ------------------------------------------------------------------
        # pre-pass: find last matmul per (stream, bank) for stop flags
        def tile_ranges(T):
            s1 = [(0, A[T], "adj"), (A[T], B[T], "m1")]
            s2 = [(A[T], B[T], "m2"), (B[T], R, "adj")]
            out = {1: [], 2: []}
            for stream, ranges in ((1, s1), (2, s2)):
                for lo, hi, src in ranges:
                    for n in range(2):
                        l = max(lo, n * 512)
                        h = min(hi, (n + 1) * 512)
                        if l < h:
                            out[stream].append((n, l, h, src))
            return out

        last_mm = {}
        for T in range(NJ):
            for stream, mms in tile_ranges(T).items():
                for k, (n, l, h, src) in enumerate(mms):
                    last_mm[(stream, n)] = (T, k)

        for q in range(NQ):
            batch = adj_pool.tile([128, 4, R], FP8, tag="adj", name=f"adj{q}")
            nc.sync.dma_start(
                out=batch[:, :, :],
                in_=bass.AP(tensor=adjT, offset=q * 512 * R,
                            ap=[[R, 128], [128 * R, 4], [1, R]]),
            )
            for s in range(4):
                T = q * 4 + s
                g, u = T // GT, T % GT
                ranges = tile_ranges(T)
                for stream, wgt, accs, mpack in (
                        (1, w1_g[g], t1_acc, m1_g[g]),
                        (2, w2_g[g], t2_acc, m2_g[g])):
                    for k, (n, l, h, src) in enumerate(ranges[stream]):
                        if src == "adj":
                            rhs = batch[:, s, l:h]
                        else:
                            rhs = mpack[:, u, l - SW[T]:h - SW[T]]
                        stop = last_mm[(stream, n)] == (T, k)
                        nc.tensor.matmul(
                            accs[n][:, l - n * 512:h - n * 512],
                            wgt[:, u, :], rhs, start=False, stop=stop)

        # ------------------------------------------------------------------
        # Tail: H = r*T1 + T2; out = elu(H[:F]/H[F])
        # ------------------------------------------------------------------
        ones65 = S.tile([1, F_OUT + 1], BF16)
        nc.vector.memset(ones65[:, :], 1.0)

        h_sb = S.tile([F_OUT + 1, R], F32)
        for n in range(2):
            sl = slice(n * 512, (n + 1) * 512)
            rb_ps = mps.tile([F_OUT + 1, 512], F32, tag="mps")
            nc.tensor.matmul(rb_ps[:, :], ones65[:, :], r_b[:, sl],
                             start=True, stop=True)
            nc.vector.tensor_copy(h_sb[:, sl], rb_ps[:, :])
            nc.vector.tensor_tensor(h_sb[:, sl], h_sb[:, sl], t1_acc[n][:, :],
                                    OP.mult)
            nc.vector.tensor_tensor(h_sb[:, sl], h_sb[:, sl], t2_acc[n][:, :],
                                    OP.add)

        zr = S.tile([1, R], F32)
        nc.vector.reciprocal(zr[:, :], h_sb[F_OUT:F_OUT + 1, :])
        zr_b = S.tile([1, R], BF16)
        nc.vector.tensor_copy(zr_b[:, :], zr[:, :])

        ot_sb = S.tile([F_OUT, R], F32)
        for n in range(2):
            sl = slice(n * 512, (n + 1) * 512)
            zb_ps = mps.tile([F_OUT, 512], F32, tag="mps")
            nc.tensor.matmul(zb_ps[:, :], ones65[:, 0:F_OUT], zr_b[:, sl],
                             start=True, stop=True)
            nc.vector.tensor_tensor(ot_sb[:, sl], h_sb[0:F_OUT, sl], zb_ps[:, :],
                                    OP.mult)

        # elu(x) = max(x,0) - 1 + exp(min(x,0))
        mn_sb = S.tile([F_OUT, R], F32)
        nc.vector.tensor_scalar(mn_sb[:, :], ot_sb[:, :], 0.0, None, OP.min)
        nc.scalar.activation(mn_sb[:, :], mn_sb[:, :], AF.Exp)
        nc.vector.tensor_scalar(ot_sb[:, :], ot_sb[:, :], 0.0, -1.0, OP.max, OP.add)
        nc.vector.tensor_tensor(ot_sb[:, :], ot_sb[:, :], mn_sb[:, :], OP.add)
        nc.sync.dma_start(out=out_d[:, :], in_=ot_sb[:, :])

    if split_waits:
        _split_multi_waits(nc)
    return nc


_CACHED = {}


def _get_compiled(A, B, S, W_S):
    key = (tuple(A), tuple(B), tuple(S), W_S)
    if key not in _CACHED:
        _CACHED.clear()
        _CACHED[key] = build_kernel(A, B, S, W_S)
    return _CACHED[key]


def _fit_window(A, B):
    """Find W_S and per-tile starts S (piecewise-arithmetic, clipped to
    [0, R-W_S]) with [S_T, S_T+W_S) covering every slab [A_T, B_T)."""
    Ts = np.arange(NJ)
    best = None
    for alpha in range(0, 49):
        beta = int((A - alpha * Ts).min())
        W = int((B - alpha * Ts).max()) - beta
        if W > 120:
            continue
        W4 = max(8, ((W + 3) // 4) * 4)
        Sv = np.clip(alpha * Ts + beta, 0, R - W4)
        if ((Sv <= A) & (B <= Sv + W4)).all():
            if best is None or W4 < best[0]:
                best = (W4, Sv)
    if best is None:
        W4 = max(8, ((int((B - A).max()) + 3) // 4) * 4)
        Sv = np.clip(A, 0, R - W4)
        best = (W4, Sv)
    W_S, Sv = best
    return [int(x) for x in Sv], int(W_S)


def kernel(t_input, o_input, W_t, W_o, a, adj, _trace=False):
    from concourse.bass_utils import run_bass_kernel_spmd

    t_input = np.asarray(t_input, dtype=np.float32)
    o_input = np.asarray(o_input, dtype=np.float32)
    W_t = np.asarray(W_t, dtype=np.float32)
    W_o = np.asarray(W_o, dtype=np.float32)
    a = np.asarray(a, dtype=np.float32)
    adj = np.asarray(adj)

    # scheduling metadata: sort j by s_o ascending, deal i by descending
    # s_t global rank round-robin across cores (so per-core split counts
    # differ by at most 1; min/max over cores shared by the SPMD program)
    s_o = (o_input @ W_o) @ a[F_OUT:, 0]
    s_t = t_input @ (W_t @ a[:F_OUT, 0])
    jperm = np.argsort(s_o, kind="stable")
    ipg = np.argsort(-s_t, kind="stable")
    so_s = s_o[jperm]
    lo = so_s[0::128]
    hi = so_s[127::128]
    A = np.full(NJ, 1 << 30, dtype=np.int64)
    Bb = np.full(NJ, -(1 << 30), dtype=np.int64)
    for m in range(N_CORES):
        neg = -s_t[ipg[m::N_CORES]]          # ascending
        Am = np.searchsorted(neg, lo, side="left")
        Bm = np.searchsorted(neg, hi, side="left")
        A = np.minimum(A, Am)
        Bb = np.maximum(Bb, Bm)
    A = np.maximum(A - 1, 0)
    Bb = np.minimum(Bb + 1, R)
    A[0] = 0
    Bb[-1] = R
    Sw, W_S = _fit_window(A, Bb)
    A = [int(x) for x in A]
    Bb = [int(x) for x in Bb]

    wt_b = W_t.astype(bf16)
    wo_b = W_o.astype(bf16)
    a_b = a.astype(bf16)
    o_Tb = np.ascontiguousarray(o_input[jperm].T).astype(bf16)

    in_maps = []
    for m in range(N_CORES):
        rows = ipg[m::N_CORES]
        adj_m = adj[rows][:, jperm].astype(np.float32)
        in_maps.append(
            {
                "t_T": np.ascontiguousarray(t_input[rows].T).astype(bf16),
                "o_T": o_Tb,
                "wt_d": wt_b,
                "wo_d": wo_b,
                "a_d": a_b,
                "adjT": np.ascontiguousarray(adj_m.T).astype(f8e4),
            }
        )

    nc = _get_compiled(A, Bb, Sw, W_S)
    res = run_bass_kernel_spmd(
        nc, in_maps, core_ids=list(range(N_CORES)), trace=_trace
    )
    out = np.empty((N_T, F_OUT), dtype=np.float32)
    for m in range(N_CORES):
        out[ipg[m::N_CORES]] = res.results[m]["out_d"].T
    if _trace:
        kernel.last_exec_time_ns = res.exec_time_ns
        kernel.last_results = res
    return out


# revision 32
# speedup vs baseline: 1.2663x; 1.0437x over previous
"""Trainium2 Bass kernel for the NodeAttentionLayer (GAT-style) problem.

Math (per reference.py):
    h_t = t_input @ W_t; h_o = o_input @ W_o
    s_t = h_t @ a[:F];  s_o = h_o @ a[F:]
    e[i,j]   = leaky_relu(s_t[i] + s_o[j], 0.2)
    att      = softmax(where(adj>0, e, -9e15), axis=1)
    out      = elu(att @ h_o)

Sorted-split identity: with c = (s_t[i]+s_o[j] > 0), v1 = exp(s_o), v2 =
exp(0.2 s_o), r = exp(0.8 s_t):
    att-numerator @ [h_o|1] = r[i] * (W1 @ M1) + (W2 @ M2)
where W1 = v1*[h_o|1], W2 = v2*[h_o|1], M1 = adj*c, M2 = adj - M1; the
ones column carries the softmax denominator; softmax max-trick cancels.

The host permutes j by ascending s_o and deals i by descending s_t rank
round-robin across the 8 cores (permutation-invariant math; output rows
un-permuted on host).  Then for every 128-j tile T, c[:,i] is all-ones
for i < A_T, all-zeros for i >= B_T, and mixed only on a narrow slab
[A_T, B_T) (~20 cols).  Prefix columns feed the W1 stream and suffix
columns the W2 stream DIRECTLY from adj (no mask work); only the slab
needs c / M1 / M2 element ops.  Each i-column crosses the PE once per
j-tile instead of twice, and the DVE/ACT mask work drops ~40x.

adj and o ship as fp8e4 (adj 0/1 is exact in fp8; mixed bf16-stationary
x fp8-moving matmul verified exact on HW), halving the dominant DMA.

Split points A_T/B_T are data-dependent; the Bass program is built per
input (compile happens inside kernel(), cached on the split tuple).
Cores share one SPMD program: A_T = min over cores, B_T = max.
"""

import contextlib
import ctypes
import sys
import types

import ml_dtypes
import numpy as np

import concourse.bass as bass
import concourse.mybir as mybir
import concourse.tile as tile
from concourse.vector_clock import ScopedClock

bf16 = ml_dtypes.bfloat16
f8e4 = ml_dtypes.float8_e4m3

# ---------------------------------------------------------------------------
# Environment shims (same as baseline)
# ---------------------------------------------------------------------------

def _patch_tile_drain():
    if getattr(tile.TileContext, "_drain_patch_installed", False):
        return

    def _drain_and_barrier(self, tick_clock, wait_clock):
        nop_inst = self.nc.sync.nop(nofuse=True)
        wait_clock.add_sem_waits(
            nop_inst.ins, ScopedClock({None: tick_clock.global_clock})
        )
        ow = list(nop_inst.ins.sync_info.on_wait) if nop_inst.ins.sync_info else []
        if len(ow) > 1:
            nop_inst.ins.sync_info.on_wait = ow[:1]
            for w in ow[1:]:
                extra = self.nc.sync.nop(nofuse=True)
                if extra.ins.sync_info is None:
                    extra.ins.sync_info = mybir.SyncInfo(on_wait=[w], on_update=[])
                else:
                    extra.ins.sync_info.on_wait = [w]
        self.nc.sync.drain()
        self.nc.all_engine_barrier()
        popped = self.nc._tile_sem_poison_stack.pop()
        assert popped is self._sem_poison
        self.nc.clear_and_free_semaphores(list(self.sems.allocated().values()))
        self.nc.all_engine_barrier()

    tile.TileContext._drain_and_barrier = _drain_and_barrier
    tile.TileContext._drain_patch_installed = True


def _install_ntff_hook():
    if "antenv.axon_hooks" in sys.modules:
        return
    import antenv

    state = {"hook": None}
    mod = types.ModuleType("antenv.axon_hooks")
    mod.set_axon_ntff_profile_hook = lambda h: state.__setitem__("hook", h)
    mod.get_axon_ntff_profile_hook = lambda: state["hook"]
    sys.modules["antenv.axon_hooks"] = mod
    antenv.axon_hooks = mod

    try:
        lib = ctypes.CDLL("/opt/axon/libaxon_pjrt.so")
    except OSError:
        return
    if not hasattr(lib, "axon_start_nrt_profile"):
        return
    lib.axon_start_nrt_profile.argtypes = [
        ctypes.POINTER(ctypes.c_int64),
        ctypes.c_size_t,
    ]
    lib.axon_start_nrt_profile.restype = ctypes.c_int64
    lib.axon_stop_nrt_profile.argtypes = [ctypes.c_char_p]
    lib.axon_stop_nrt_profile.restype = ctypes.c_int64

    @contextlib.contextmanager
    def _ntff_hook(output_dir, device_ids):
        import jax

        jax.devices()
        if device_ids:
            ids = (ctypes.c_int64 * len(device_ids))(*device_ids)
            rc = lib.axon_start_nrt_profile(ids, len(device_ids))
        else:
            rc = lib.axon_start_nrt_profile(None, 0)
        if rc != 0:
            raise RuntimeError(f"axon_start_nrt_profile rc={rc}")
        try:
            yield
        finally:
            n = lib.axon_stop_nrt_profile(str(output_dir).encode())
            print(f"profile: {n} file(s) written to {output_dir}", file=sys.stderr)

    state["hook"] = _ntff_hook


_patch_tile_drain()
_install_ntff_hook()

# Walrus disables its LDWEIGHTS optimizer by default; each self-loading
# matmul then pays its weight-load serially (~90ns). Flipping the flag lets
# consecutive matmuls overlap weight loads. Toggleable for A/B testing.
LDW_OPT = False   # --enable-ldw-opt=true breaks walrus visitInstLdweights


def _install_ldw_opt_patch():
    import concourse.bass_utils as _bu

    if getattr(_bu, "_ldw_opt_patch", False):
        return
    _orig = _bu.run_command

    def _patched(cmd, *args, **kw):
        if LDW_OPT and isinstance(cmd, list):
            cmd = ["--enable-ldw-opt=true" if c == "--enable-ldw-opt=false" else c
                   for c in cmd]
        return _orig(cmd, *args, **kw)

    _bu.run_command = _patched
    _bu._ldw_opt_patch = True


_install_ldw_opt_patch()


def _split_multi_waits(nc):
    import bass_rust

    k = 0
    for f in nc.m.functions:
        for blk in f.blocks:
            insts = blk.instructions
            out = []
            changed = False
            for inst in insts:
                si = inst.sync_info
                ow = list(si.on_wait) if si is not None else []
                if len(ow) > 1:
                    for w in ow[:-1]:
                        nop = bass_rust.InstNoOp(
                            name=f"waitsplit-{k}", engine=inst.engine
                        )
                        k += 1
                        nop.sync_info = mybir.SyncInfo(on_wait=[w], on_update=[])
                        out.append(nop)
                    si.on_wait = [ow[-1]]
                    changed = True
                out.append(inst)
            if changed:
                blk.instructions = out


# ---------------------------------------------------------------------------
# Problem constants
# ---------------------------------------------------------------------------
N_T, N_O, F_IN, F_OUT = 8192, 8192, 256, 64
N_CORES = 8
R = N_T // N_CORES            # 1024 t-rows per core
NJ = N_O // 128               # 64 j-tiles
NQ = NJ // 4                  # 16 quads (adj DMA batches of 4 tiles)
KC = F_IN // 128              # 2 contraction chunks
NG = 4                        # o-side groups
GT = NJ // NG                 # 16 j-tiles per group
F32 = mybir.dt.float32
BF16 = mybir.dt.bfloat16
FP8 = mybir.dt.float8e4
AF = mybir.ActivationFunctionType
OP = mybir.AluOpType


def _rep_free(ap, reps):
    """View [P, K] tile as [P, K, reps] via innermost step-0."""
    return bass.AP(tensor=ap.tensor, offset=ap.offset, ap=list(ap.ap) + [[0, reps]])


def build_kernel(A, B, SW, W_S, split_waits=True):
    """A, B: length-64 int tuples; j-tile T uses raw adj on i in [0,A_T)
    (stream 1) and [B_T,1024) (stream 2); masked slab on [A_T,B_T).
    SW: per-tile start of the W_S-wide adj window ([S_T, S_T+W_S) covers
    the slab); S is piecewise-arithmetic in T so the window loads in a
    handful of constant-stride DMAs."""
    assert W_S <= 128

    nc = bass.Bass("TRN2")

    t_T = nc.dram_tensor("t_T", [F_IN, R], BF16, kind="ExternalInput")
    o_T = nc.dram_tensor("o_T", [F_IN, N_O], BF16, kind="ExternalInput")
    wt_d = nc.dram_tensor("wt_d", [F_IN, F_OUT], BF16, kind="ExternalInput")
    wo_d = nc.dram_tensor("wo_d", [F_IN, F_OUT], BF16, kind="ExternalInput")
    a_d = nc.dram_tensor("a_d", [2 * F_OUT, 1], BF16, kind="ExternalInput")
    adjT = nc.dram_tensor("adjT", [N_O, R], FP8, kind="ExternalInput")
    adj_win = nc.dram_tensor("adj_win", [128, NJ * W_S], FP8,
                             kind="ExternalInput")
    out_d = nc.dram_tensor("out_d", [F_OUT, R], F32, kind="ExternalOutput")
    dbg_d = nc.dram_tensor("dbg_d", [2, R], F32, kind="ExternalOutput")

    st_dram = nc.dram_tensor("st_dram", [1, R], BF16, kind="Internal")
    hoT_dram_g = [nc.dram_tensor(f"hoT_dram{g}", [80, 2048], BF16,
                                 kind="Internal") for g in range(NG)]

    with tile.TileContext(nc) as tc, contextlib.ExitStack() as ctx:
        S = ctx.enter_context(tc.tile_pool(name="singles", bufs=1))
        adj_pool = ctx.enter_context(tc.tile_pool(name="adj", bufs=4))
        acc = ctx.enter_context(tc.tile_pool(name="acc", bufs=1, space="PSUM"))
        mps = ctx.enter_context(tc.tile_pool(name="mps", bufs=2, space="PSUM"))
        hop = ctx.enter_context(tc.tile_pool(name="hop", bufs=2, space="PSUM"))

        # ------------------------------------------------------------------
        # t-side head
        # ------------------------------------------------------------------
        wt_sb = S.tile([128, KC, F_OUT], BF16)
        nc.sync.dma_start(
            out=wt_sb[:, :, :],
            in_=bass.AP(tensor=wt_d, offset=0,
                        ap=[[F_OUT, 128], [128 * F_OUT, KC], [1, F_OUT]]))
        wo_sb = S.tile([128, KC, F_OUT], BF16)
        nc.sync.dma_start(
            out=wo_sb[:, :, :],
            in_=bass.AP(tensor=wo_d, offset=0,
                        ap=[[F_OUT, 128], [128 * F_OUT, KC], [1, F_OUT]]))
        a_t_b = S.tile([128, F_OUT], BF16)
        nc.sync.dma_start(
            out=a_t_b[:, :],
            in_=bass.AP(tensor=a_d, offset=0, ap=[[0, 128], [1, F_OUT]]))
        a_o_b = S.tile([128, F_OUT], BF16)
        nc.sync.dma_start(
            out=a_o_b[:, :],
            in_=bass.AP(tensor=a_d, offset=F_OUT, ap=[[0, 128], [1, F_OUT]]))
        t_sb = S.tile([128, KC, R], BF16)
        nc.sync.dma_start(
            out=t_sb[:, :, :],
            in_=bass.AP(tensor=t_T, offset=0,
                        ap=[[R, 128], [128 * R, KC], [1, R]]))

        # w065 = [W_o | W_o @ a_o]  (projects o -> [h_o | s_o])
        w065 = S.tile([128, KC, F_OUT + 1], BF16)
        nc.vector.tensor_copy(w065[:, :, 0:F_OUT], wo_sb[:, :, :])
        prod_ao = S.tile([128, KC, F_OUT], F32)
        ao_ap = a_o_b[:, :]
        nc.vector.tensor_tensor(
            prod_ao[:, :, :], wo_sb[:, :, :],
            bass.AP(tensor=ao_ap.tensor, offset=ao_ap.offset,
                    ap=[list(ao_ap.ap[0]), [0, KC], [1, F_OUT]]),
            OP.mult)
        woa_f = S.tile([128, KC], F32)
        nc.vector.tensor_reduce(woa_f[:, :], prod_ao[:, :, :],
                                mybir.AxisListType.X, OP.add)
        nc.vector.tensor_copy(w065[:, :, F_OUT], woa_f[:, :])

        # wta = W_t @ a_t (so s_t = t @ wta without materializing h_t)
        prod_at = S.tile([128, KC, F_OUT], F32)
        at_ap = a_t_b[:, :]
        nc.vector.tensor_tensor(
            prod_at[:, :, :], wt_sb[:, :, :],
            bass.AP(tensor=at_ap.tensor, offset=at_ap.offset,
                    ap=[list(at_ap.ap[0]), [0, KC], [1, F_OUT]]),
            OP.mult)
        wta_f = S.tile([128, KC], F32)
        nc.vector.tensor_reduce(wta_f[:, :], prod_at[:, :, :],
                                mybir.AxisListType.X, OP.add)
        wta_b = S.tile([128, KC], BF16)
        nc.vector.tensor_copy(wta_b[:, :], wta_f[:, :])

        # zero moving buffer: 4 full-width start=True matmuls clear the
        # accumulator banks' has_written for the whole bank (robust across
        # repeated NEFF executions with partial-coverage accumulation).
        # Dependency-free (memset weights) so they fire early and warm HAM.
        t1_acc = [acc.tile([F_OUT + 1, 512], F32, tag=f"t1_{n}", name=f"t1_{n}")
                  for n in range(2)]
        t2_acc = [acc.tile([F_OUT + 1, 512], F32, tag=f"t2_{n}", name=f"t2_{n}")
                  for n in range(2)]
        with tc.high_priority():
            zbuf = S.tile([128, 512], FP8)
            nc.vector.memset(zbuf[:, :], 0.0)
            zw = S.tile([128, F_OUT + 1], BF16)
            nc.vector.memset(zw[:, :], 0.0)
            ones65 = S.tile([1, F_OUT + 1], BF16)
            nc.vector.memset(ones65[:, :], 1.0)
            ones_f = S.tile([1, F_OUT], F32)
            nc.vector.memset(ones_f[:, :], 1.0)
            # dep-free HAM warmers: fill the head PE idle window so the
            # st/hoT matmuls run at 2.4GHz (results discarded - each
            # start=True re-clears the bank; real zero-MMs follow)
            for w in range(8):
                nc.tensor.matmul(t1_acc[w % 2][:, :], zw[:, :], zbuf[:, :],
                                 start=True, stop=(w >= 6))
            for ps in (t1_acc[0], t1_acc[1], t2_acc[0], t2_acc[1]):
                nc.tensor.matmul(ps[:, :], zw[:, :], zbuf[:, :],
                                 start=True, stop=False)

        # s_t and r = exp(0.8 s_t)
        with tc.high_priority():
            st_b = S.tile([1, R], BF16)
            r_b = S.tile([1, R], BF16)
            for n in range(R // 512):
                st_ps = mps.tile([1, 512], F32, tag="mps")
                for c in range(KC):
                    nc.tensor.matmul(st_ps[:, :], wta_b[:, c:c + 1],
                                     t_sb[:, c, n * 512:(n + 1) * 512],
                                     start=(c == 0), stop=(c == KC - 1))
                nc.vector.tensor_copy(st_b[:, n * 512:(n + 1) * 512], st_ps[:, :])
                nc.scalar.activation(r_b[:, n * 512:(n + 1) * 512], st_ps[:, :],
                                     AF.Exp, scale=0.8)
            nc.sync.dma_start(out=st_dram[:, :], in_=st_b[0:1, :])
            st_bcast = S.tile([128, R], BF16)
            nc.sync.dma_start(
                out=st_bcast[:, :],
                in_=bass.AP(tensor=st_dram, offset=0, ap=[[0, 128], [1, R]]),
            )

        # ------------------------------------------------------------------
        # o-side, group-pipelined: h_oT, s_o, transpose, W-build, slabs
        # ------------------------------------------------------------------
        ho_g_t = [S.tile([128, GT, 80], BF16, name=f"hog{g}") for g in range(NG)]
        nso_g = [S.tile([128, GT], F32, name=f"nsog{g}") for g in range(NG)]
        v1_g = [S.tile([128, GT], BF16, name=f"v1g{g}") for g in range(NG)]
        v2_g = [S.tile([128, GT], BF16, name=f"v2g{g}") for g in range(NG)]
        w1_g = [S.tile([128, GT, F_OUT + 1], BF16, name=f"w1g{g}")
                for g in range(NG)]
        w2_g = [S.tile([128, GT, F_OUT + 1], BF16, name=f"w2g{g}")
                for g in range(NG)]
        adjs = S.tile([128, NJ, W_S], FP8, name="adjs")
        c_g = [S.tile([128, GT, W_S], BF16, name=f"cg{g}") for g in range(NG)]
        m1_g = [S.tile([128, GT, W_S], BF16, name=f"m1g{g}") for g in range(NG)]
        m2_g = [S.tile([128, GT, W_S], BF16, name=f"m2g{g}") for g in range(NG)]

        # o DMAs upfront (4 live buffers); adj slab window split per group
        # into constant-stride runs (few DMA instructions, ~2us of
        # descriptor-gen each, interleaved so no single 8us ring block)
        o_gs = []
        for g in range(NG):
            o_g = S.tile([128, KC, 2048], BF16, name=f"og{g}")
            nc.scalar.dma_start(
                out=o_g[:, :, :],
                in_=bass.AP(tensor=o_T, offset=g * 2048,
                            ap=[[N_O, 128], [128 * N_O, KC], [1, 2048]]))
            o_gs.append(o_g)
        # adj slab window: host pre-gathers [128, NJ, W_S] partition-major,
        # so this is one DMA with contiguous 3KB-per-partition rows (the
        # naive strided gather floods the queues with 8192 48B descriptors)
        nc.sync.dma_start(
            out=adjs[:, :, :],
            in_=bass.AP(tensor=adj_win, offset=0,
                        ap=[[NJ * W_S, 128], [W_S, NJ], [1, W_S]]))

        hoT_bufs = [S.tile([80, 2048], BF16, name=f"hoth{k}") for k in range(2)]

        def emit_group_prep(g):
            o_g = o_gs[g]
            hoT_g = hoT_bufs[g % 2]
            for ch in range(4):
                hps = hop.tile([F_OUT + 1, 512], F32, tag="hop",
                               name=f"hop{g}_{ch}")
                for c in range(KC):
                    nc.tensor.matmul(
                        hps[:, :], w065[:, c, :],
                        o_g[:, c, ch * 512:(ch + 1) * 512],
                        start=(c == 0), stop=(c == KC - 1))
                nc.scalar.copy(
                    hoT_g[0:F_OUT + 1, ch * 512:(ch + 1) * 512], hps[:, :])
            nc.scalar.dma_start(out=hoT_dram_g[g][:, :], in_=hoT_g[:, :])
            nc.scalar.dma_start_transpose(ho_g_t[g][:, :, :], hoT_dram_g[g][:, :])

            so_view = ho_g_t[g][:, :, F_OUT]
            nc.vector.tensor_scalar(nso_g[g][:, :], so_view, -1.0, None, OP.mult)
            nc.scalar.activation(v1_g[g][:, :], so_view, AF.Exp)
            nc.scalar.activation(v2_g[g][:, :], so_view, AF.Exp, scale=0.2)

            # W-build
            nc.vector.tensor_tensor(
                w1_g[g][:, :, 0:F_OUT], ho_g_t[g][:, :, 0:F_OUT],
                _rep_free(v1_g[g][:, :], F_OUT), OP.mult)
            nc.vector.tensor_copy(w1_g[g][:, :, F_OUT], v1_g[g][:, :])
            nc.vector.tensor_tensor(
                w2_g[g][:, :, 0:F_OUT], ho_g_t[g][:, :, 0:F_OUT],
                _rep_free(v2_g[g][:, :], F_OUT), OP.mult)
            nc.vector.tensor_copy(w2_g[g][:, :, F_OUT], v2_g[g][:, :])

            # slab prep: c = (s_t > -s_o) on [A,B) inside the window; M1; M2
            nc.vector.memset(c_g[g][:, :, :], 0.0)
            for u in range(GT):
                T = g * GT + u
                w = B[T] - A[T]
                if w <= 0:
                    continue
                off = A[T] - SW[T]
                nc.vector.tensor_scalar(
                    c_g[g][:, u, off:off + w], st_bcast[:, A[T]:B[T]],
                    nso_g[g][:, u:u + 1], None, OP.is_gt)
            gsl = slice(g * GT, (g + 1) * GT)
            nc.vector.tensor_tensor(m1_g[g][:, :, :], c_g[g][:, :, :],
                                    adjs[:, gsl, :], OP.mult)
            nc.vector.tensor_tensor(m2_g[g][:, :, :], adjs[:, gsl, :],
                                    m1_g[g][:, :, :], OP.subtract)

        # ------------------------------------------------------------------
        # Main: per j-tile, prefix/suffix from raw adj + slab from M1/M2.
        # Group prep interleaves with the main loop: prep for group g+2 is
        # emitted after group g's quads so its ACT/DVE/DMA chain overlaps
        # main-loop matmuls instead of stalling them.
        # ------------------------------------------------------------------
        # pre-pass: find last matmul per (stream, bank) for stop flags
        def tile_ranges(T):
            s1 = [(0, A[T], "adj"), (A[T], B[T], "m1")]
            s2 = [(A[T], B[T], "m2"), (B[T], R, "adj")]
            out = {1: [], 2: []}
            for stream, ranges in ((1, s1), (2, s2)):
                for lo, hi, src in ranges:
                    for n in range(2):
                        l = max(lo, n * 512)
                        h = min(hi, (n + 1) * 512)
                        if l < h:
                            out[stream].append((n, l, h, src))
            return out

        last_mm = {}
        for q in range(NQ):
            for phase in ("adj", "m"):
                for s in range(4):
                    T = q * 4 + s
                    for stream, mms in tile_ranges(T).items():
                        for k, (n, l, h, src) in enumerate(mms):
                            if (src == "adj") != (phase == "adj"):
                                continue
                            last_mm[(stream, n)] = (T, phase, k)

        emit_group_prep(0)
        emit_group_prep(1)
        for gq in range(NG):
            for q in range(gq * (NQ // NG), (gq + 1) * (NQ // NG)):
                batch = adj_pool.tile([128, 4, R], FP8, tag="adj", name=f"adj{q}")
                nc.sync.dma_start(
                    out=batch[:, :, :],
                    in_=bass.AP(tensor=adjT, offset=q * 512 * R,
                                ap=[[R, 128], [128 * R, 4], [1, R]]),
                )
                for phase in ("adj", "m"):
                    for s in range(4):
                        T = q * 4 + s
                        g, u = T // GT, T % GT
                        ranges = tile_ranges(T)
                        for stream, wgt, accs, mpack in (
                                (1, w1_g[g], t1_acc, m1_g[g]),
                                (2, w2_g[g], t2_acc, m2_g[g])):
                            for k, (n, l, h, src) in enumerate(ranges[stream]):
                                if (src == "adj") != (phase == "adj"):
                                    continue
                                if src == "adj":
                                    rhs = batch[:, s, l:h]
                                else:
                                    rhs = mpack[:, u, l - SW[T]:h - SW[T]]
                                stop = last_mm[(stream, n)] == (T, phase, k)
                                nc.tensor.matmul(
                                    accs[n][:, l - n * 512:h - n * 512],
                                    wgt[:, u, :], rhs, start=False, stop=stop)
            if gq + 2 < NG:
                emit_group_prep(gq + 2)

        # ------------------------------------------------------------------
        # Tail: H = r*T1 + T2; out = elu(H[:F]/H[F])
        # ------------------------------------------------------------------
        h_sb = S.tile([F_OUT + 1, R], F32)
        for n in range(2):
            sl = slice(n * 512, (n + 1) * 512)
            rb_ps = mps.tile([F_OUT + 1, 512], F32, tag="mps")
            nc.tensor.matmul(rb_ps[:, :], ones65[:, :], r_b[:, sl],
                             start=True, stop=True)
            nc.vector.tensor_copy(h_sb[:, sl], rb_ps[:, :])
            nc.vector.tensor_tensor(h_sb[:, sl], h_sb[:, sl], t1_acc[n][:, :],
                                    OP.mult)
            nc.vector.tensor_tensor(h_sb[:, sl], h_sb[:, sl], t2_acc[n][:, :],
                                    OP.add)

        zr = S.tile([1, R], F32)
        zl = S.tile([1, R], F32)
        nc.scalar.activation(zl[:, :], h_sb[F_OUT:F_OUT + 1, :], AF.Ln)
        nc.scalar.activation(zr[:, :], zl[:, :], AF.Exp, scale=-1.0)
        nc.sync.dma_start(out=dbg_d[0:1, :], in_=h_sb[F_OUT:F_OUT + 1, :])
        nc.sync.dma_start(out=dbg_d[1:2, :], in_=zr[0:1, :])
        ot_sb = S.tile([F_OUT, R], F32)
        for n in range(2):
            sl = slice(n * 512, (n + 1) * 512)
            zb_ps = mps.tile([F_OUT, 512], F32, tag="mps")
            nc.tensor.matmul(zb_ps[:, :], ones_f[:, :], zr[:, sl],
                             start=True, stop=True)
            nc.vector.tensor_tensor(ot_sb[:, sl], h_sb[0:F_OUT, sl], zb_ps[:, :],
                                    OP.mult)

        # elu(x) = max(x,0) - 1 + exp(min(x,0)); per-half so the first
        # output DMA overlaps the second half's element ops
        mn_sb = S.tile([F_OUT, R], F32)
        for n in range(2):
            sl = slice(n * 512, (n + 1) * 512)
            nc.vector.tensor_scalar(mn_sb[:, sl], ot_sb[:, sl], 0.0, None, OP.min)
            nc.scalar.activation(mn_sb[:, sl], mn_sb[:, sl], AF.Exp)
            nc.vector.tensor_scalar(ot_sb[:, sl], ot_sb[:, sl], 0.0, -1.0,
                                    OP.max, OP.add)
            nc.vector.tensor_tensor(ot_sb[:, sl], ot_sb[:, sl], mn_sb[:, sl],
                                    OP.add)
            nc.sync.dma_start(out=out_d[:, sl], in_=ot_sb[:, sl])

    if split_waits:
        _split_multi_waits(nc)
    return nc


_CACHED = {}


def _get_compiled(A, B, S, W_S):
    key = (tuple(A), tuple(B), tuple(S), W_S)
    if key not in _CACHED:
        _CACHED.clear()
        _CACHED[key] = build_kernel(A, B, S, W_S)
    return _CACHED[key]


def _fit_window(A, B):
    """W_S = max slab width (padded); window starts tight at A_T, clipped
    so [S_T, S_T+W_S) stays in [0, R]."""
    W4 = max(8, ((int((B - A).max()) + 3) // 4) * 4)
    Sv = np.clip(A, 0, R - W4)
    return [int(x) for x in Sv], int(W4)


def kernel(t_input, o_input, W_t, W_o, a, adj, _trace=False):
    from concourse.bass_utils import run_bass_kernel_spmd

    t_input = np.asarray(t_input, dtype=np.float32)
    o_input = np.asarray(o_input, dtype=np.float32)
    W_t = np.asarray(W_t, dtype=np.float32)
    W_o = np.asarray(W_o, dtype=np.float32)
    a = np.asarray(a, dtype=np.float32)
    adj = np.asarray(adj)

    # scheduling metadata: sort j by s_o ascending, deal i by descending
    # s_t global rank round-robin across cores (so per-core split counts
    # differ by at most 1; min/max over cores shared by the SPMD program)
    s_o = (o_input @ W_o) @ a[F_OUT:, 0]
    s_t = t_input @ (W_t @ a[:F_OUT, 0])
    jperm = np.argsort(s_o, kind="stable")
    ipg = np.argsort(-s_t, kind="stable")
    so_s = s_o[jperm]
    lo = so_s[0::128]
    hi = so_s[127::128]
    A = np.full(NJ, 1 << 30, dtype=np.int64)
    Bb = np.full(NJ, -(1 << 30), dtype=np.int64)
    for m in range(N_CORES):
        neg = -s_t[ipg[m::N_CORES]]          # ascending
        Am = np.searchsorted(neg, lo, side="left")
        Bm = np.searchsorted(neg, hi, side="left")
        A = np.minimum(A, Am)
        Bb = np.maximum(Bb, Bm)
    A = np.maximum(A - 1, 0)
    Bb = np.minimum(Bb + 1, R)
    A[0] = 0
    Bb[-1] = R
    Sw, W_S = _fit_window(A, Bb)
    A = [int(x) for x in A]
    Bb = [int(x) for x in Bb]

    wt_b = W_t.astype(bf16)
    wo_b = W_o.astype(bf16)
    a_b = a.astype(bf16)
    o_Tb = np.ascontiguousarray(o_input[jperm].T).astype(bf16)

    in_maps = []
    for m in range(N_CORES):
        rows = ipg[m::N_CORES]
        adj_m = adj[rows][:, jperm].astype(np.float32)
        adjT_m = np.ascontiguousarray(adj_m.T).astype(f8e4)
        win = np.empty((128, NJ, W_S), dtype=f8e4)
        for T in range(NJ):
            win[:, T, :] = adjT_m[128 * T:128 * (T + 1), Sw[T]:Sw[T] + W_S]
        in_maps.append(
            {
                "t_T": np.ascontiguousarray(t_input[rows].T).astype(bf16),
                "o_T": o_Tb,
                "wt_d": wt_b,
                "wo_d": wo_b,
                "a_d": a_b,
                "adjT": adjT_m,
                "adj_win": win.reshape(128, NJ * W_S),
            }
        )

    nc = _get_compiled(A, Bb, Sw, W_S)
    res = run_bass_kernel_spmd(
        nc, in_maps, core_ids=list(range(N_CORES)), trace=_trace
    )
    out = np.empty((N_T, F_OUT), dtype=np.float32)
    for m in range(N_CORES):
        out[ipg[m::N_CORES]] = res.results[m]["out_d"].T
    if _trace:
        kernel.last_exec_time_ns = res.exec_time_ns
        kernel.last_results = res
    return out
